# revision 2
# baseline (speedup 1.0000x reference)
"""Trainium2 Bass kernel for nn_DenseAttentionOneHead.

reference:  q = X @ W^T ; pre = q @ X^T ; out = pre @ X     (X [2,4096,1024])
All linear, so out_b = (X_b W^T)(X_b^T X_b) = X_b (W^T S_b) with S_b = X_b^T X_b.

Sharding (8 cores): cores 0-3 handle batch 0, cores 4-7 batch 1; each core owns
1024 rows of its batch.  Per core:
  S_part = Xs^T Xs                    (fp32r matmuls, fp32 PSUM)
  S      = AllReduce(S_part) over the 4-core group   ([D,D] fp32)
  At     = Xs^T  (PE transposes; overlaps the AllReduce)
  M      = W^T S
  out    = Xs M  (via lhsT = At blocks)
"""

import numpy as np

import concourse.mybir as mybir
import concourse.tile as tile
from concourse import bacc
from concourse.bass_utils import run_bass_kernel_spmd
from concourse.masks import make_identity

F32 = mybir.dt.float32
F32R = mybir.dt.float32r
P = 128
D = 1024
B = 2
N = 4096
NCORES = 8
GROUP = 4          # cores per batch
ROWS = N // GROUP  # 1024 rows per core
NO = D // P        # 8
RO = ROWS // P     # 8 row-chunks per core

_compiled = None


def _build():
    nc = bacc.Bacc(None, target_bir_lowering=False, debug=False, num_devices=NCORES)

    xs = nc.dram_tensor("xs", [ROWS, D], F32, kind="ExternalInput")
    w = nc.dram_tensor("w", [D, D], F32, kind="ExternalInput")
    o_out = nc.dram_tensor("o_out", [ROWS, D], F32, kind="ExternalOutput")

    s_bounce = nc.dram_tensor("s_bounce", [D, D], F32)
    s_red = nc.dram_tensor("s_red", [D, D], F32)

    with tile.TileContext(nc) as tc:
        with (
            tc.tile_pool(name="big", bufs=1) as big,
            tc.tile_pool(name="stage", bufs=4) as stage,
            tc.tile_pool(name="psum", bufs=4, space="PSUM") as psum,
            tc.tile_pool(name="psum_t", bufs=4, space="PSUM") as psum_t,
        ):
            A = big.tile([P, RO, D], F32R, tag="A")
            Wt = big.tile([P, NO, D], F32R, tag="W")
            S = big.tile([P, NO, D], F32R, tag="S")
            M = big.tile([P, NO, D], F32R, tag="M")
            At = big.tile([P, NO, ROWS], F32R, tag="At")

            ident_f = stage.tile([P, P], F32, tag="ident_f")
            make_identity(nc, ident_f)
            ident = stage.tile([P, P], F32R, tag="ident")
            nc.vector.tensor_copy(ident[:], ident_f[:])

            nc.sync.dma_start(A[:], xs[:].rearrange("(no p) d -> p no d", p=P).bitcast(F32R))
            nc.sync.dma_start(Wt[:], w[:].rearrange("(no p) d -> p no d", p=P).bitcast(F32R))

            # ---- S_part = Xs^T Xs : [e, d], contraction over this core's rows
            for et in range(NO):
                for h in range(2):
                    acc = psum.tile([P, 512], F32, tag="acc")
                    for ch in range(RO):
                        nc.tensor.matmul(
                            acc[:],
                            A[:, ch, et * P : (et + 1) * P],
                            A[:, ch, h * 512 : (h + 1) * 512],
                            start=(ch == 0),
                            stop=(ch == RO - 1),
                        )
                    st = stage.tile([P, 512], F32, tag="st")
                    nc.vector.tensor_copy(st[:], acc[:])
                    nc.sync.dma_start(
                        s_bounce[et * P : (et + 1) * P, h * 512 : (h + 1) * 512], st[:]
                    )

            # ---- AllReduce S over the 4-core group
            nc.gpsimd.collective_compute(
                "AllReduce",
                mybir.AluOpType.add,
                replica_groups=[[0, 1, 2, 3], [4, 5, 6, 7]],
                ins=[s_bounce[:].opt()],
                outs=[s_red[:].opt()],
            )
            nc.sync.dma_start(
                S[:], s_red[:].rearrange("(no p) d -> p no d", p=P).bitcast(F32R)
            )

            # ---- At = Xs^T (overlaps the AllReduce; no dep on S)
            for no in range(RO):
                for do in range(NO):
                    pt = psum_t.tile([P, P], F32R, tag="pt")
                    nc.tensor.transpose(pt[:], A[:, no, do * P : (do + 1) * P], ident[:])
                    nc.scalar.copy(At[:, do, no * P : (no + 1) * P], pt[:])

            # ---- M = W^T S : M[a, d] = sum_e W[e, a] S[e, d]
            for at in range(NO):
                for h in range(2):
                    acc = psum.tile([P, 512], F32, tag="acc")
                    for ch in range(NO):
                        nc.tensor.matmul(
                            acc[:],
                            Wt[:, ch, at * P : (at + 1) * P],
                            S[:, ch, h * 512 : (h + 1) * 512],
                            start=(ch == 0),
                            stop=(ch == NO - 1),
                        )
                    nc.vector.tensor_copy(M[:, at, h * 512 : (h + 1) * 512], acc[:])

            # ---- out = Xs M : lhsT = At blocks, rhs = M
            for nt in range(RO):
                for h in range(2):
                    acc = psum.tile([P, 512], F32, tag="acc")
                    for ch in range(NO):
                        nc.tensor.matmul(
                            acc[:],
                            At[:, ch, nt * P : (nt + 1) * P],
                            M[:, ch, h * 512 : (h + 1) * 512],
                            start=(ch == 0),
                            stop=(ch == NO - 1),
                        )
                    ot = stage.tile([P, 512], F32, tag="ot")
                    nc.vector.tensor_copy(ot[:], acc[:])
                    nc.sync.dma_start(
                        o_out[:].rearrange("(no p) d -> p no d", p=P)[
                            :, nt, h * 512 : (h + 1) * 512
                        ],
                        ot[:],
                    )

    nc.finalize()
    return nc


def _get_compiled():
    global _compiled
    if _compiled is None:
        _compiled = _build()
    return _compiled


def kernel(hidden_states, queries, _trace=False, _trace_cores=None):
    x = np.ascontiguousarray(np.asarray(hidden_states, dtype=np.float32))
    w = np.ascontiguousarray(np.asarray(queries, dtype=np.float32))
    assert x.shape == (B, N, D) and w.shape == (D, D)

    nc = _get_compiled()
    in_maps = []
    for c in range(NCORES):
        b, r = c // GROUP, c % GROUP
        in_maps.append({"xs": x[b, r * ROWS : (r + 1) * ROWS], "w": w})

    res = run_bass_kernel_spmd(
        nc,
        in_maps,
        core_ids=list(range(NCORES)),
        trace=_trace,
        trace_cores=_trace_cores,
    )

    out = np.empty((B, N, D), dtype=np.float32)
    for c in range(NCORES):
        b, r = c // GROUP, c % GROUP
        out[b, r * ROWS : (r + 1) * ROWS] = res.results[c]["o_out"]

    if _trace:
        kernel.last_result = res
    return out


# revision 3
# speedup vs baseline: 1.3514x; 1.3514x over previous
"""Trainium2 Bass kernel for nn_DenseAttentionOneHead.

reference:  q = X @ W^T ; pre = q @ X^T ; out = pre @ X     (X [2,4096,1024])
All linear, so out_b = (X_b W^T)(X_b^T X_b) = Q_b S_b with
  Q_b = X_b W^T,  S_b = X_b^T X_b  ([D,D], summed over rows -> AllReduce).

Sharding (8 cores): cores 0-3 handle batch 0, cores 4-7 batch 1; each core owns
1024 rows of its batch.  Per core (Plan A — fill the AllReduce window):
  S_part = Xs^T Xs             -> fp16 -> AllReduce (4-core group)
  Wt^T, Xs^T  (PE transposes)  \  run while the AllReduce is in flight,
  Q^T = W Xs^T                 /  keeping the PE busy and warm
  out = Q S   (lhsT = Q^T blocks, rhs = S)
"""

import numpy as np

import concourse.mybir as mybir
import concourse.tile as tile
from concourse import bacc
from concourse.bass_utils import run_bass_kernel_spmd
from concourse.masks import make_identity

F32 = mybir.dt.float32
F32R = mybir.dt.float32r
F16 = mybir.dt.float16
P = 128
D = 1024
B = 2
N = 4096
NCORES = 8
GROUP = 4          # cores per batch
ROWS = N // GROUP  # 1024 rows per core
NO = D // P        # 8
RO = ROWS // P     # 8 row-chunks per core

_compiled = None


def _build():
    nc = bacc.Bacc(None, target_bir_lowering=False, debug=False, num_devices=NCORES)

    xs = nc.dram_tensor("xs", [ROWS, D], F32, kind="ExternalInput")
    w = nc.dram_tensor("w", [D, D], F32, kind="ExternalInput")
    o_out = nc.dram_tensor("o_out", [ROWS, D], F32, kind="ExternalOutput")

    s_bounce = nc.dram_tensor("s_bounce", [D, D], F16)
    s_red = nc.dram_tensor("s_red", [D, D], F16)

    with tile.TileContext(nc) as tc:
        with (
            tc.tile_pool(name="big", bufs=1) as big,
            tc.tile_pool(name="wstage", bufs=3) as wstage,
            tc.tile_pool(name="stage", bufs=4) as stage,
            tc.tile_pool(name="psum", bufs=4, space="PSUM") as psum,
            tc.tile_pool(name="psum_t", bufs=4, space="PSUM") as psum_t,
        ):
            A = big.tile([P, RO, D], F32R, tag="A")        # Xs, row-chunk layout
            WT = big.tile([P, NO, D], F32R, tag="WT")      # W^T  [d, e]
            At = big.tile([P, NO, ROWS], F32R, tag="At")   # Xs^T [d, n]
            Qt = big.tile([P, NO, ROWS], F32R, tag="Qt")   # Q^T  [e, n]
            S = big.tile([P, NO, D], F32R, tag="A")        # reuses A's buffer

            ident_f = stage.tile([P, P], F32, tag="ident_f")
            make_identity(nc, ident_f)
            ident = stage.tile([P, P], F32R, tag="ident")
            nc.vector.tensor_copy(ident[:], ident_f[:])

            # Per-chunk loads so the first matmuls start early
            for ch in range(RO):
                nc.sync.dma_start(
                    A[:, ch, :],
                    xs[ch * P : (ch + 1) * P, :].bitcast(F32R),
                )

            # ---- S_part = Xs^T Xs : [e, d]; cast fp16 and bounce to DRAM
            for et in range(NO):
                for h in range(2):
                    acc = psum.tile([P, 512], F32, tag="acc")
                    for ch in range(RO):
                        nc.tensor.matmul(
                            acc[:],
                            A[:, ch, et * P : (et + 1) * P],
                            A[:, ch, h * 512 : (h + 1) * 512],
                            start=(ch == 0),
                            stop=(ch == RO - 1),
                        )
                    sh = stage.tile([P, 512], F16, tag="sh")
                    nc.vector.tensor_copy(sh[:], acc[:])
                    nc.sync.dma_start(
                        s_bounce[et * P : (et + 1) * P, h * 512 : (h + 1) * 512], sh[:]
                    )

            # ---- AllReduce S over the 4-core group (fp16 payload)
            nc.gpsimd.collective_compute(
                "AllReduce",
                mybir.AluOpType.add,
                replica_groups=[[0, 1, 2, 3], [4, 5, 6, 7]],
                ins=[s_bounce[:].opt()],
                outs=[s_red[:].opt()],
            )

            # ---- W^T and Xs^T transposes + Q^T = W Xs^T (fill the AR window)
            for eo in range(NO):
                wst = wstage.tile([P, D], F32R, tag="wst")
                nc.sync.dma_start(
                    wst[:], w[eo * P : (eo + 1) * P, :].bitcast(F32R)
                )
                for do in range(NO):
                    pt = psum_t.tile([P, P], F32R, tag="pt")
                    nc.tensor.transpose(pt[:], wst[:, do * P : (do + 1) * P], ident[:])
                    nc.scalar.copy(WT[:, do, eo * P : (eo + 1) * P], pt[:])
            for no in range(RO):
                for do in range(NO):
                    pt = psum_t.tile([P, P], F32R, tag="pt")
                    nc.tensor.transpose(pt[:], A[:, no, do * P : (do + 1) * P], ident[:])
                    nc.scalar.copy(At[:, do, no * P : (no + 1) * P], pt[:])

            # Q^T[e, n] = sum_d W[e, d] Xs[n, d] : lhsT = W^T blocks, rhs = Xs^T
            for et in range(NO):
                for h in range(2):
                    acc = psum.tile([P, 512], F32, tag="acc")
                    for ch in range(NO):
                        nc.tensor.matmul(
                            acc[:],
                            WT[:, ch, et * P : (et + 1) * P],
                            At[:, ch, h * 512 : (h + 1) * 512],
                            start=(ch == 0),
                            stop=(ch == NO - 1),
                        )
                    nc.vector.tensor_copy(Qt[:, et, h * 512 : (h + 1) * 512], acc[:])

            # ---- S back from the collective, upcast fp16 -> fp32r
            for eo in range(NO):
                sr = stage.tile([P, D], F16, tag="sr")
                nc.sync.dma_start(sr[:], s_red[eo * P : (eo + 1) * P, :])
                nc.vector.tensor_copy(S[:, eo, :], sr[:])

            # ---- out = Q S : lhsT = Q^T blocks, rhs = S
            for nt in range(RO):
                for h in range(2):
                    acc = psum.tile([P, 512], F32, tag="acc")
                    for ch in range(NO):
                        nc.tensor.matmul(
                            acc[:],
                            Qt[:, ch, nt * P : (nt + 1) * P],
                            S[:, ch, h * 512 : (h + 1) * 512],
                            start=(ch == 0),
                            stop=(ch == NO - 1),
                        )
                    ot = stage.tile([P, 512], F32, tag="ot")
                    nc.vector.tensor_copy(ot[:], acc[:])
                    nc.sync.dma_start(
                        o_out[nt * P : (nt + 1) * P, h * 512 : (h + 1) * 512], ot[:]
                    )

    nc.finalize()
    return nc


def _get_compiled():
    global _compiled
    if _compiled is None:
        _compiled = _build()
    return _compiled


def kernel(hidden_states, queries, _trace=False, _trace_cores=None):
    x = np.ascontiguousarray(np.asarray(hidden_states, dtype=np.float32))
    w = np.ascontiguousarray(np.asarray(queries, dtype=np.float32))
    assert x.shape == (B, N, D) and w.shape == (D, D)

    nc = _get_compiled()
    in_maps = []
    for c in range(NCORES):
        b, r = c // GROUP, c % GROUP
        in_maps.append({"xs": x[b, r * ROWS : (r + 1) * ROWS], "w": w})

    res = run_bass_kernel_spmd(
        nc,
        in_maps,
        core_ids=list(range(NCORES)),
        trace=_trace,
        trace_cores=_trace_cores,
    )

    out = np.empty((B, N, D), dtype=np.float32)
    for c in range(NCORES):
        b, r = c // GROUP, c % GROUP
        out[b, r * ROWS : (r + 1) * ROWS] = res.results[c]["o_out"]

    if _trace:
        kernel.last_result = res
    return out
